# revision 8
# baseline (speedup 1.0000x reference)
"""LocalCorrelation (13x13 cost volume) Trainium2 kernel.

Full inputs z_t, z_t1: [8, 256, 128, 128] f32 -> out [8, 169, 128, 128] f32.
out[b, 13*di+dj, h, w] = sum_c z_t[b,c,h,w] * pad(z_t1)[b,c,h+di,w+dj] / 16

Sharding: data-parallel over batch, 1 batch element per NeuronCore (8 cores).
Host pre-processing (free): cast to bf16, block-major reorder of z_t,
zero-pad of z_t1.  Host post-processing: upcast bf16 output to f32.

Per-core pipeline (h-halves hh=0,1 roll through SBUF so the hh=0 gather
overlaps the hh=1 compute):
  - Rolling input tiles per quarter hq: z_t1 padded rows
    [32*hq, 32*hq+44), z_t block-major quarter [4096].  ~70 warm-up
    matmuls bridge the HAM clock-gate through the initial load.
  - Per 8x16 pixel block: block-gram matmuls, stationary = z_t block
    [c, 128 pix], streaming = padded z_t1 20x28 window in two 280-col
    halves -> one [128,1024] PSUM tile (halves at cols 0 / 512),
    accumulated over the 2 c-chunks; ONE fused PSUM->SBUF evacuation
    per block with the 1/16 scale; ONE dense 2-dim DMA per stripe into
    DRAM scratch (per-hh tensor):
      addr = 573664*(si%8) + 4480*pix + 560*wb + (28*p + q)
  - Superset gather per hh, 16 DMAs with 728B runs:
      o2[h, (dw, wb, s)] = scr[hh][71708*(h%64) + 4480*dw + 560*wb + s]
    71708 = 16*4480 + 28 makes the per-h read start absorb the
    diagonal: s = 28*di + dw + dj.
  - Per di: one 4-dim strided copy -> o4[h, (dj, w)] (the dj-shear is
    affine in free dims only), one DMA -> out[di*13+dj, h, w].
"""

import numpy as np

C = 256
H = W = 128
KS = 13
KK = 169
RAD = 6
HP = WP = 140
SA = 8
SB = 16
NWB = 8
NST = 16
WINQ = 28
WIN = 560              # 20 * 28
PIXS = 4480            # scratch pixel stride (= 8*560, dense)
WBS = 560
DHS = 16 * PIXS + 28   # 71708
SIS = 8 * DHS          # 573664
XBW = 8 * WBS          # 4480
RUN = 364              # 13 * 28 superset run
O2W = 128 * RUN        # 46592
O4W = KS * W           # 1664
Z1QR = 44              # padded z1 rows resident per h-quarter

_cache = {}


def _build():
    import concourse.bass as bass
    import concourse.mybir as mybir
    import concourse.tile as tile
    from concourse import bacc

    f32 = mybir.dt.float32
    bf16 = mybir.dt.bfloat16

    nc = bacc.Bacc("TRN2", target_bir_lowering=False, debug=False)
    zt_d = nc.dram_tensor("zt", [2, 128, H * W], bf16, kind="ExternalInput")
    z1_d = nc.dram_tensor("z1p", [2, 128, HP * WP], bf16, kind="ExternalInput")
    out_d = nc.dram_tensor("out", [KK, H, W], bf16, kind="ExternalOutput")

    with tile.TileContext(nc) as tc:
        with (
            tc.tile_pool(name="scrp", bufs=1, space="DRAM") as scrp,
            tc.tile_pool(name="pp", bufs=1) as pp,
            tc.tile_pool(name="z1qp", bufs=2) as z1qp,
            tc.tile_pool(name="ztqp", bufs=2) as ztqp,
            tc.tile_pool(name="xbp", bufs=2) as xbp,
            tc.tile_pool(name="psp", bufs=4, space="PSUM") as psp,
            tc.tile_pool(name="o2p", bufs=1) as o2p,
            tc.tile_pool(name="o4p", bufs=2) as o4p,
        ):
            scr = [scrp.tile([8, SIS], bf16, tag=f"scr{hh}", name=f"scr{hh}")
                   for hh in range(2)]
            o2 = o2p.tile([128, O2W], bf16, tag="o2", name="o2")

            # PE warm-up while the first loads are in flight
            wt = pp.tile([128, 640], bf16, tag="wt", name="wt")
            nc.vector.memset(wt[:, :], 0.0)
            for _ in range(70):
                wps = psp.tile([128, 1024], f32, tag="ps", name="ps")
                nc.tensor.matmul(wps[:, 0:512], wt[:, 0:128], wt[:, 128:640],
                                 start=True, stop=True)

            Z1Q = {}
            ZTQ = {}

            def emit_loads(hq):
                z1 = [z1qp.tile([128, Z1QR * WP], bf16, tag=f"z1q{k}", name=f"z1q{k}")
                      for k in range(2)]
                zt = [ztqp.tile([128, 4096], bf16, tag=f"ztq{k}", name=f"ztq{k}")
                      for k in range(2)]
                r0 = 32 * hq * WP
                nc.sync.dma_start(z1[0][:, :], z1_d.ap()[0][:, r0:r0 + Z1QR * WP])
                nc.scalar.dma_start(z1[1][:, :], z1_d.ap()[1][:, r0:r0 + Z1QR * WP])
                b0 = hq * 4096
                nc.sync.dma_start(zt[0][:, :], zt_d.ap()[0][:, b0:b0 + 4096])
                nc.scalar.dma_start(zt[1][:, :], zt_d.ap()[1][:, b0:b0 + 4096])
                Z1Q[hq], ZTQ[hq] = z1, zt

            def emit_stripes(hq):
                for sl in range(4):
                    h0l = sl * SA   # stripe row base, local to the quarter
                    xb = xbp.tile([128, XBW], bf16, tag="xb", name="xb")
                    for wb in range(NWB):
                        w0 = wb * SB
                        blkl = sl * NWB + wb
                        ps = psp.tile([128, 1024], f32, tag="ps", name="ps")
                        for k in range(2):
                            lhsT = ZTQ[hq][k][:, blkl * 128:(blkl + 1) * 128]
                            for half in range(2):
                                rhs = Z1Q[hq][k].rearrange("c (h w) -> c h w", h=Z1QR)[
                                    :, h0l + 10 * half: h0l + 10 * (half + 1),
                                    w0:w0 + WINQ]
                                nc.tensor.matmul(
                                    ps[:, half * 512: half * 512 + 280], lhsT, rhs,
                                    start=(k == 0), stop=(k == 1))
                        esrc = bass.AP(ps.tensor, 0, [[1024, 128], [512, 2], [1, 280]])
                        edst = bass.AP(xb.tensor, wb * WBS, [[XBW, 128], [280, 2], [1, 280]])
                        if wb % 2 == 0:
                            nc.scalar.mul(edst, esrc, 1.0 / 16.0)
                        else:
                            nc.vector.tensor_scalar_mul(edst, esrc, 1.0 / 16.0)
                    sg = hq // 2                      # scratch half tensor
                    sl_g = (hq % 2) * 4 + sl          # stripe within the half
                    w_dst = bass.AP(scr[sg].tensor, sl_g * SIS, [[PIXS, 128], [1, XBW]])
                    nc.gpsimd.dma_start(w_dst, xb[:, :])

            def emit_reads(hh, spread):
                for dw in range(16):
                    rsrc = bass.AP(scr[hh].tensor, PIXS * dw,
                                   [[DHS, 64], [WBS, 8], [1, RUN]])
                    rdst = bass.AP(o2.tensor, 64 * hh * O2W + dw * 8 * RUN,
                                   [[O2W, 64], [RUN, 8], [1, RUN]])
                    eng = (nc.sync if dw % 2 == 0 else nc.scalar) if spread else nc.sync
                    eng.dma_start(rdst, rsrc)

            emit_loads(0)
            emit_loads(1)
            emit_stripes(0)
            emit_loads(2)
            emit_stripes(1)
            emit_loads(3)
            emit_reads(0, spread=False)   # overlaps quarters 2-3, sync queue only
            emit_stripes(2)
            emit_stripes(3)
            emit_reads(1, spread=True)

            for di in range(KS):
                o4 = o4p.tile([128, O4W], bf16, tag="o4", name="o4")
                csrc = bass.AP(o2.tensor, 28 * di,
                               [[O2W, 128], [1, KS], [RUN, 8], [8 * RUN + 1, 16]])
                cdst = bass.AP(o4.tensor, 0,
                               [[O4W, 128], [W, KS], [16, 8], [1, 16]])
                if di % 2 == 0:
                    nc.vector.tensor_copy(cdst, csrc)
                else:
                    nc.scalar.copy(cdst, csrc)
                ow_dst = bass.AP(out_d, di * KS * H * W,
                                 [[W, 128], [H * W, KS], [1, W]])
                ow_src = bass.AP(o4.tensor, 0, [[O4W, 128], [W, KS], [1, W]])
                eng = nc.sync if di % 2 == 0 else nc.scalar
                eng.dma_start(ow_dst, ow_src)

    nc.compile()
    return nc


def _get_nc():
    if "nc" not in _cache:
        _cache["nc"] = _build()
    return _cache["nc"]


def _prep(z_t: np.ndarray, z_t1: np.ndarray):
    """Host-side: cast to bf16, block-major reorder z_t, pad z_t1."""
    import ml_dtypes
    bf = ml_dtypes.bfloat16
    zt = z_t.astype(bf).reshape(2, 128, NST, SA, NWB, SB)
    zt = np.ascontiguousarray(zt.transpose(0, 1, 2, 4, 3, 5)).reshape(2, 128, H * W)
    z1 = np.pad(z_t1.astype(bf), ((0, 0), (RAD, RAD), (RAD, RAD)))
    z1 = np.ascontiguousarray(z1).reshape(2, 128, HP * WP)
    return zt, z1


def kernel(z_t: np.ndarray, z_t1: np.ndarray) -> np.ndarray:
    from concourse.bass_utils import run_bass_kernel_spmd

    nc = _get_nc()
    B = z_t.shape[0]
    in_maps = []
    for i in range(B):
        zt, z1 = _prep(z_t[i], z_t1[i])
        in_maps.append({"zt": zt, "z1p": z1})
    res = run_bass_kernel_spmd(nc, in_maps, core_ids=list(range(B)))
    return np.stack(
        [np.asarray(res.results[i]["out"]).astype(np.float32) for i in range(B)],
        axis=0)


# revision 10
# speedup vs baseline: 1.0547x; 1.0547x over previous
"""LocalCorrelation (13x13 cost volume) Trainium2 kernel.

Full inputs z_t, z_t1: [8, 256, 128, 128] f32 -> out [8, 169, 128, 128] f32.
out[b, 13*di+dj, h, w] = sum_c z_t[b,c,h,w] * pad(z_t1)[b,c,h+di,w+dj] / 16

Sharding: data-parallel over batch, 1 batch element per NeuronCore (8 cores).
Host pre-processing (free): cast to bf16, block-major reorder of z_t,
zero-pad of z_t1.  Host post-processing: upcast bf16 output to f32.

Per-core pipeline, rolling h-quarters (hq = 4 stripes = 32 rows):
  - Rolling input tiles per quarter: z_t1 padded rows [32*hq, 32*hq+44),
    z_t block-major quarter [4096].  ~70 warm-up matmuls bridge the HAM
    clock-gate through the initial load.
  - Per 8x16 pixel block: block-gram matmuls, stationary = z_t block
    [c, 128 pix], streaming = padded z_t1 20x28 window in two 280-col
    halves -> one [128,1024] PSUM tile (halves at cols 0 / 512),
    accumulated over the 2 c-chunks; ONE fused PSUM->SBUF evacuation
    per block with the 1/16 scale; ONE dense 2-dim DMA per stripe into
    the quarter's DRAM scratch tensor:
      addr = 573664*sl + 4480*pix + 560*wb + (28*p + q)
  - Superset gather per quarter (overlaps the next quarter's compute),
    16 DMAs each with 728B runs:
      o2[h, (dw, wb, s)] = scr[hq][71708*(h%32) + 4480*dw + 560*wb + s]
    71708 = 16*4480 + 28 makes the per-h read start absorb the
    diagonal: s = 28*di + dw + dj.
  - Per (h-half, di): one 4-dim strided copy -> o4[h, (dj, w)] (the
    dj-shear is affine in free dims only), one DMA -> out.  The h-half
    0 copies run during the last quarter's compute.
"""

import numpy as np

C = 256
H = W = 128
KS = 13
KK = 169
RAD = 6
HP = WP = 140
SA = 8
SB = 16
NWB = 8
NST = 16
WINQ = 28
WIN = 560              # 20 * 28
PIXS = 4480            # scratch pixel stride (= 8*560, dense)
WBS = 560
DHS = 16 * PIXS + 28   # 71708
SIS = 8 * DHS          # 573664
XBW = 8 * WBS          # 4480
RUN = 364              # 13 * 28 superset run
O2W = 128 * RUN        # 46592
O4W = KS * W           # 1664
Z1QR = 44              # padded z1 rows resident per h-quarter

_cache = {}


def _build():
    import concourse.bass as bass
    import concourse.mybir as mybir
    import concourse.tile as tile
    from concourse import bacc

    f32 = mybir.dt.float32
    bf16 = mybir.dt.bfloat16

    nc = bacc.Bacc("TRN2", target_bir_lowering=False, debug=False)
    zt_d = nc.dram_tensor("zt", [2, 128, H * W], bf16, kind="ExternalInput")
    z1_d = nc.dram_tensor("z1p", [2, 128, HP * WP], bf16, kind="ExternalInput")
    out_d = nc.dram_tensor("out", [KK, H, W], bf16, kind="ExternalOutput")

    with tile.TileContext(nc) as tc:
        with (
            tc.tile_pool(name="scrp", bufs=1, space="DRAM") as scrp,
            tc.tile_pool(name="pp", bufs=1) as pp,
            tc.tile_pool(name="z1qp", bufs=2) as z1qp,
            tc.tile_pool(name="ztqp", bufs=2) as ztqp,
            tc.tile_pool(name="xbp", bufs=2) as xbp,
            tc.tile_pool(name="psp", bufs=4, space="PSUM") as psp,
            tc.tile_pool(name="o2p", bufs=1) as o2p,
            tc.tile_pool(name="o4p", bufs=3) as o4p,
        ):
            scr = [scrp.tile([4, SIS], bf16, tag=f"scr{q}", name=f"scr{q}")
                   for q in range(4)]
            o2 = o2p.tile([128, O2W], bf16, tag="o2", name="o2")

            # PE warm-up while the first loads are in flight
            wt = pp.tile([128, 640], bf16, tag="wt", name="wt")
            nc.vector.memset(wt[:, :], 0.0)
            for _ in range(70):
                wps = psp.tile([128, 1024], f32, tag="ps", name="ps")
                nc.tensor.matmul(wps[:, 0:512], wt[:, 0:128], wt[:, 128:640],
                                 start=True, stop=True)

            Z1Q = {}
            ZTQ = {}

            def emit_loads(hq):
                z1 = [z1qp.tile([128, Z1QR * WP], bf16, tag=f"z1q{k}", name=f"z1q{k}")
                      for k in range(2)]
                zt = [ztqp.tile([128, 4096], bf16, tag=f"ztq{k}", name=f"ztq{k}")
                      for k in range(2)]
                r0 = 32 * hq * WP
                nc.sync.dma_start(z1[0][:, :], z1_d.ap()[0][:, r0:r0 + Z1QR * WP])
                nc.scalar.dma_start(z1[1][:, :], z1_d.ap()[1][:, r0:r0 + Z1QR * WP])
                b0 = hq * 4096
                nc.sync.dma_start(zt[0][:, :], zt_d.ap()[0][:, b0:b0 + 4096])
                nc.scalar.dma_start(zt[1][:, :], zt_d.ap()[1][:, b0:b0 + 4096])
                Z1Q[hq], ZTQ[hq] = z1, zt

            def emit_stripe(hq, sl):
                h0l = sl * SA   # stripe row base, local to the quarter
                xb = xbp.tile([128, XBW], bf16, tag="xb", name="xb")
                for wb in range(NWB):
                    w0 = wb * SB
                    blkl = sl * NWB + wb
                    ps = psp.tile([128, 1024], f32, tag="ps", name="ps")
                    for k in range(2):
                        lhsT = ZTQ[hq][k][:, blkl * 128:(blkl + 1) * 128]
                        for half in range(2):
                            rhs = Z1Q[hq][k].rearrange("c (h w) -> c h w", h=Z1QR)[
                                :, h0l + 10 * half: h0l + 10 * (half + 1),
                                w0:w0 + WINQ]
                            nc.tensor.matmul(
                                ps[:, half * 512: half * 512 + 280], lhsT, rhs,
                                start=(k == 0), stop=(k == 1))
                    esrc = bass.AP(ps.tensor, 0, [[1024, 128], [512, 2], [1, 280]])
                    edst = bass.AP(xb.tensor, wb * WBS, [[XBW, 128], [280, 2], [1, 280]])
                    if wb % 2 == 0:
                        nc.scalar.mul(edst, esrc, 1.0 / 16.0)
                    else:
                        nc.vector.tensor_scalar_mul(edst, esrc, 1.0 / 16.0)
                w_dst = bass.AP(scr[hq].tensor, sl * SIS, [[PIXS, 128], [1, XBW]])
                nc.gpsimd.dma_start(w_dst, xb[:, :])

            def emit_reads(hq, spread=False):
                for dw in range(16):
                    rsrc = bass.AP(scr[hq].tensor, PIXS * dw,
                                   [[DHS, 32], [WBS, 8], [1, RUN]])
                    rdst = bass.AP(o2.tensor, 32 * hq * O2W + dw * 8 * RUN,
                                   [[O2W, 32], [RUN, 8], [1, RUN]])
                    eng = (nc.sync if dw % 2 == 0 else nc.scalar) if spread else nc.sync
                    eng.dma_start(rdst, rsrc)

            def cast_op(hh, di, which):
                o4 = o4p.tile([64, O4W], bf16, tag="o4", name="o4")
                csrc = bass.AP(o2.tensor, 64 * hh * O2W + 28 * di,
                               [[O2W, 64], [1, KS], [RUN, 8], [8 * RUN + 1, 16]])
                cdst = bass.AP(o4.tensor, 0,
                               [[O4W, 64], [W, KS], [16, 8], [1, 16]])
                if which == 0:
                    nc.vector.tensor_copy(cdst, csrc)
                elif which == 1:
                    nc.scalar.copy(cdst, csrc)
                else:
                    nc.gpsimd.tensor_copy(cdst, csrc)
                ow_dst = bass.AP(out_d, di * KS * H * W + 64 * hh * W,
                                 [[W, 64], [H * W, KS], [1, W]])
                ow_src = bass.AP(o4.tensor, 0, [[O4W, 64], [W, KS], [1, W]])
                oeng = nc.sync if di % 2 == 0 else nc.scalar
                oeng.dma_start(ow_dst, ow_src)

            emit_loads(0)
            emit_loads(1)
            for sl in range(4):
                emit_stripe(0, sl)
            emit_loads(2)
            emit_reads(0)
            for sl in range(4):
                emit_stripe(1, sl)
            emit_loads(3)
            emit_reads(1)
            for sl in range(4):
                emit_stripe(2, sl)
            emit_reads(2)
            # quarter 3 compute, with h-half 0 permute copies interleaved
            for sl in range(4):
                emit_stripe(3, sl)
                for j in range(2):
                    di = sl * 2 + j
                    cast_op(0, di, di % 2)
            for di in range(8, KS):
                cast_op(0, di, di % 2)
            emit_reads(3, spread=True)
            for di in range(KS):
                cast_op(1, di, di % 3)

    nc.compile()
    return nc


def _get_nc():
    if "nc" not in _cache:
        _cache["nc"] = _build()
    return _cache["nc"]


def _prep(z_t: np.ndarray, z_t1: np.ndarray):
    """Host-side: cast to bf16, block-major reorder z_t, pad z_t1."""
    import ml_dtypes
    bf = ml_dtypes.bfloat16
    zt = z_t.astype(bf).reshape(2, 128, NST, SA, NWB, SB)
    zt = np.ascontiguousarray(zt.transpose(0, 1, 2, 4, 3, 5)).reshape(2, 128, H * W)
    z1 = np.pad(z_t1.astype(bf), ((0, 0), (RAD, RAD), (RAD, RAD)))
    z1 = np.ascontiguousarray(z1).reshape(2, 128, HP * WP)
    return zt, z1


def kernel(z_t: np.ndarray, z_t1: np.ndarray) -> np.ndarray:
    from concourse.bass_utils import run_bass_kernel_spmd

    nc = _get_nc()
    B = z_t.shape[0]
    in_maps = []
    for i in range(B):
        zt, z1 = _prep(z_t[i], z_t1[i])
        in_maps.append({"zt": zt, "z1p": z1})
    res = run_bass_kernel_spmd(nc, in_maps, core_ids=list(range(B)))
    return np.stack(
        [np.asarray(res.results[i]["out"]).astype(np.float32) for i in range(B)],
        axis=0)
